# revision 54
# baseline (speedup 1.0000x reference)
"""Tensor-parallel MultiHeadAttention kernel for 8 Trainium2 NeuronCores.

Problem (hardcoded): B=2, N=2048, C=1024, H=16 heads, D=64.
Sharding: core c handles the head PAIR {2c, 2c+1} for BOTH batches.
Out-projection contracts only this core's 128 features, so each core
emits a full-shape partial per batch; the host sums the 8 partials and
adds the output bias.

Schedule (single pass, software-pipelined):
  prologue: Q(b0), K(b0) projection
  4 head-units x 16 kb units: scores -> exp (ScalarE-paced); each unit
  also carries the PREVIOUS head's AV qb-pass + normalize, plus paced
  injections: V blocks, QKV(b1), outproj(b0).
  tail: AV/normalize/transpose/outproj for the last head (b1).

AV uses the transposed formulation: out[128q, 64] += pt^T V with the
exp'd scores as the PE stationary operand (free dim 64, not 512) --
half the PE cycles of the V-stationary form.  Softmax denominators come
from near-free free-dim=1 ones-matmuls sharing the same stationary.
o^T transposes ride the SP DMA xbar; psum->sbuf copies and Q/K bias
adds run on DVE (plus idle ScalarE in the tail) so ScalarE mostly does
exp.

A tuned subset of the exp tiles (DVE_EXP) runs on DVE instead of
ScalarE using a Schraudolph-style fast exp: i16 = trunc(A*s + B) whose
bits form the bf16 representation of ~exp(s).  B is tuned for zero
mass-weighted mean bias so mixed exact/approx softmax rows stay
consistent; the residual sawtooth is ~1.8% RMS on ~15% of the
attention mass, well inside the accuracy budget.  Normalize (paired
reciprocal+broadcast-multiply), o^T transposes, V-block copies,
out-projection and output DMAs are all batched in qb-pairs to halve
instruction dispatch overheads; the final pair issues per-block DMAs
so the last transfer overlaps the last copies.
"""

import sys

import numpy as np
import ml_dtypes

try:
    import concourse.bass  # noqa: F401
except ImportError:
    for p in ("/opt/trn_rl_repo", "/root/.axon_site/_ro/trn_rl_repo"):
        if p not in sys.path:
            sys.path.insert(0, p)

B, N, C, H, D = 2, 2048, 1024, 16, 64
NCORES = 8
HPC = 2            # heads per core
DL = HPC * D       # 128 local feature dim
NB = N // 128      # 16 token/key blocks
QC = N // 512      # 4 tok chunks for projection

# Schraudolph bf16 fast-exp constants: bits(bf16(~exp(s))) = trunc(A*s + B)
SCHRAUD_A = 128.0 / float(np.log(2.0))
SCHRAUD_B = 16249.25

# exp tiles offloaded to DVE: {(unit, kb, half)} -- placed in windows where
# ScalarE paces the schedule but DVE has slack
DVE_EXP = ({(0, kb, half) for kb in (3, 7) for half in (0, 1)}
           | {(1, kb, half) for kb in (3, 7) for half in (0, 1)}
           | {(2, kb, half) for kb in (5, 9) for half in (0, 1)}
           | {(3, kb, 0) for kb in (1, 3, 5, 7, 9, 11, 13)})

_cache: dict = {}
_probe_log: list = []


def _mk_probe(nc, enabled):
    if not enabled:
        return lambda eng, label, ap=None: None
    from concourse.bass_interp import add_callback2

    def probe(eng, label, ap=None):
        ins = [ap] if ap is not None else []
        add_callback2(eng, lambda sim, inst, label=label: _probe_log.append(
            (label, float(getattr(sim, "time", -1)))), ins)

    return probe


def _patch_drain_cap():
    """The walrus build in this container rejects instructions carrying
    more than a couple of sync-wait commands.  Split excess waits onto
    same-engine NoOps emitted just before the offending instruction."""
    import concourse.mybir as mybir
    from concourse.tile import TileContext
    from concourse.vector_clock import ScopedClock

    if getattr(TileContext, "_drain_cap_patched", False):
        return
    CAP = 1

    orig_commit = TileContext._commit_instruction

    def commit_split(self, inst, lazy_reg_writes=True):
        si = getattr(inst, "sync_info", None)
        if si is not None and si.on_wait is not None and len(si.on_wait) > CAP:
            waits = list(si.on_wait)
            keep = waits[len(waits) - CAP:]
            extra = waits[:len(waits) - CAP]
            for i in range(0, len(extra), CAP):
                nop = mybir.InstNoOp(
                    name=self.nc.get_next_instruction_name(),
                    engine=inst.engine,
                    sync_info=mybir.SyncInfo(on_wait=extra[i:i + CAP],
                                             on_update=[]),
                    bass_nofuse=True,
                )
                orig_commit(self, nop, lazy_reg_writes)
            inst.sync_info = mybir.SyncInfo(
                on_wait=keep, on_update=list(si.on_update))
        return orig_commit(self, inst, lazy_reg_writes)

    TileContext._commit_instruction = commit_split

    def patched(self, tick_clock, wait_clock):
        nc = self.nc
        drain_inst = nc.sync.drain()
        wait_clock.add_sem_waits(
            drain_inst.ins, ScopedClock({None: tick_clock.global_clock})
        )
        si = drain_inst.ins.sync_info
        if si is not None and len(si.on_wait) > CAP:
            waits = list(si.on_wait)
            drain_inst.ins.sync_info = mybir.SyncInfo(
                on_wait=waits[:CAP], on_update=list(si.on_update)
            )
            for i in range(CAP, len(waits), CAP):
                nop_bi = nc.sync.nop(nofuse=True)
                nop_bi.ins.sync_info = mybir.SyncInfo(
                    on_wait=waits[i : i + CAP], on_update=[]
                )
        nc.all_engine_barrier()
        assert self.sems is not None
        popped = nc._tile_sem_poison_stack.pop()
        assert popped is self._sem_poison
        nc.clear_and_free_semaphores(list(self.sems.allocated().values()))
        nc.all_engine_barrier()

    TileContext._drain_and_barrier = patched
    TileContext._drain_cap_patched = True


def _build():
    import os
    import concourse.bass as bass
    import concourse.mybir as mybir
    from concourse.tile import TileContext
    from contextlib import ExitStack

    _patch_drain_cap()

    f32 = mybir.dt.float32
    bf16 = mybir.dt.bfloat16
    i16 = mybir.dt.int16
    AF = mybir.ActivationFunctionType
    ALU = mybir.AluOpType

    probing = os.environ.get("BASS_PROBE") == "1"
    nc = bass.Bass(debug=True) if probing else bass.Bass()
    probe = _mk_probe(nc, probing)
    xt_p = [nc.declare_dram_parameter(f"xt{b}", [C, N], bf16, isOutput=False)
            for b in range(B)]
    wq_p = nc.declare_dram_parameter("wqT", [C, DL], bf16, isOutput=False)
    wk_p = nc.declare_dram_parameter("wkT", [C, DL], bf16, isOutput=False)
    wv_p = nc.declare_dram_parameter("wvT", [C, DL], bf16, isOutput=False)
    wo_p = nc.declare_dram_parameter("woT", [DL, C], bf16, isOutput=False)
    bq_p = nc.declare_dram_parameter("bq", [128, 1], f32, isOutput=False)
    bk_p = nc.declare_dram_parameter("bk", [128, 1], f32, isOutput=False)
    out_p = [nc.declare_dram_parameter(f"out{b}", [N, C], bf16, isOutput=True)
             for b in range(B)]

    with TileContext(nc) as tc, ExitStack() as ctx:
        # ---- long-lived SBUF pools ----
        wpool = ctx.enter_context(tc.tile_pool(name="w", bufs=1))
        xpool = ctx.enter_context(tc.tile_pool(name="x", bufs=38))
        qpool = ctx.enter_context(tc.tile_pool(name="q", bufs=2))
        kpool = ctx.enter_context(tc.tile_pool(name="k", bufs=2))
        vpool = ctx.enter_context(tc.tile_pool(name="v", bufs=2))
        ptpool = ctx.enter_context(tc.tile_pool(name="pt", bufs=54))
        opool = ctx.enter_context(tc.tile_pool(name="o", bufs=2))
        otpool = ctx.enter_context(tc.tile_pool(name="ot", bufs=2))
        obpool = ctx.enter_context(tc.tile_pool(name="ob", bufs=3))
        recpool = ctx.enter_context(tc.tile_pool(name="rec", bufs=2))

        # ---- weights / constants into SBUF ----
        # wq rides the Pool queue first (Q chunks need it immediately);
        # the rest queue behind the first x chunks
        wq_sb = wpool.tile([128, 8 * 128], bf16, tag="wq")
        wk_sb = wpool.tile([128, 8 * 128], bf16, tag="wk")
        wv_sb = wpool.tile([128, 8 * 128], bf16, tag="wv")
        wo_sb = wpool.tile([128, C], bf16, tag="wo")

        def load_w(wsb, wp):
            nc.gpsimd.dma_start(
                out=wsb.rearrange("p (c d) -> p c d", c=8),
                in_=wp.rearrange("(c p) d -> p c d", p=128))

        load_w(wq_sb, wq_p)
        bq_sb = wpool.tile([128, 1], f32, tag="bq")
        bk_sb = wpool.tile([128, 1], f32, tag="bk")
        ones_sb = wpool.tile([128, 1], bf16, tag="ones")
        nc.vector.memset(ones_sb[:], 1.0)
        # PE p-state warm-up: keep the PE continuously busy on dummy work
        # while the first x tiles stream in, so real matmuls start at the
        # fully-ramped clock
        wu_sb = wpool.tile([128, 512], bf16, tag="wu")
        nc.vector.memset(wu_sb[:], 0.0)
        # V with a trailing all-ones column per (block, head): one AV matmul
        # then also accumulates the softmax denominator into column 64
        vone = {}

        def vone_ap(b, kb, h):
            return vone[b][:, (kb * HPC + h) * 65:(kb * HPC + h) * 65 + 65]

        # ---- x tiles: [128 c-chunk, 512 tok] ----
        xt = {}

        def load_x(b, cc, ch, eng):
            t = xpool.tile([128, 512], bf16, tag="xt", name=f"x{b}_{cc}_{ch}")
            eng.dma_start(out=t[:],
                          in_=xt_p[b][cc * 128:(cc + 1) * 128,
                                      ch * 512:(ch + 1) * 512])
            xt[(b, cc, ch)] = t

        # b0 x: split across SP/Act(/Pool) DMA queues (all idle pre-exp);
        # ch0 avoids the Pool queue, which is busy with wq
        qs = (nc.sync, nc.scalar, nc.gpsimd)
        for ch in range(QC):
            for cc in range(8):
                eng = qs[cc % 2] if ch == 0 else qs[cc % 3]
                load_x(0, cc, ch, eng)
            if ch == 0:
                load_w(wk_sb, wk_p)
                nc.gpsimd.dma_start(out=bq_sb[:], in_=bq_p[:])
                nc.gpsimd.dma_start(out=bk_sb[:], in_=bk_p[:])
            elif ch == 1:
                load_w(wv_sb, wv_p)
            elif ch == 2:
                nc.gpsimd.dma_start(out=wo_sb[:], in_=wo_p[:])

        # ---- per-batch projection tiles ----
        QT, KT, VS = {}, {}, {}

        def alloc_proj(b):
            QT[b] = qpool.tile([128, N], bf16, tag="qt", name=f"QT{b}")
            KT[b] = kpool.tile([128, N], bf16, tag="kt", name=f"KT{b}")
            VS[b] = vpool.tile([128, NB * HPC * 65], bf16, tag="vs",
                               name=f"VS{b}")
            vone[b] = VS[b]
            nc.vector.memset(
                VS[b].rearrange("p (g e) -> p g e", e=65)[:, :, 64:65], 1.0)

        alloc_proj(0)

        def emit_q_chunk(b, ch, pool, tag):
            ps = pool.tile([128, 512], f32, tag=tag, name=f"q{b}_{ch}")
            for cc in range(8):
                nc.tensor.matmul(ps[:], wq_sb[:, cc * 128:(cc + 1) * 128],
                                 xt[(b, cc, ch)][:],
                                 start=(cc == 0), stop=(cc == 7))
            nc.vector.tensor_scalar_add(
                QT[b][:, ch * 512:(ch + 1) * 512], ps[:], bq_sb[:, 0:1])

        def emit_k_chunk(b, ch, pool, tag):
            ps = pool.tile([128, 512], f32, tag=tag, name=f"k{b}_{ch}")
            for cc in range(8):
                nc.tensor.matmul(ps[:], wk_sb[:, cc * 128:(cc + 1) * 128],
                                 xt[(b, cc, ch)][:],
                                 start=(cc == 0), stop=(cc == 7))
            nc.vector.tensor_scalar_add(
                KT[b][:, ch * 512:(ch + 1) * 512], ps[:], bk_sb[:, 0:1])

        def emit_v_block(b, nb, pool, tag):
            ps = pool.tile([128, 512], f32, tag=tag, name=f"v{b}_{nb}")
            for cc in range(8):
                nc.tensor.matmul(
                    ps[:, 0:128],
                    xt[(b, cc, nb // 4)][:, (nb % 4) * 128:(nb % 4 + 1) * 128],
                    wv_sb[:, cc * 128:(cc + 1) * 128],
                    start=(cc == 0), stop=(cc == 7))
            # one strided copy fills both heads' 64-col groups
            nc.vector.tensor_copy(
                VS[b].rearrange("p (g e) -> p g e", e=65)
                     [:, 2 * nb:2 * nb + 2, 0:64],
                ps[:, 0:128].rearrange("p (g e) -> p g e", e=64))

        # ---- main phase ----
        with tc.tile_pool(name="ps", bufs=2, space="PSUM") as psp, \
             tc.tile_pool(name="av", bufs=2, space="PSUM") as avp, \
             tc.tile_pool(name="sh", bufs=2, space="PSUM") as shp:

            # PE warm-up: ~3us of dummy matmuls on memset data so the
            # p-state ramp completes while the first x tiles stream in;
            # a dummy exp pre-loads the activation table off-critical-path
            wu_ps = shp.tile([1, 512], f32, tag="sh", name="wu_ps")
            for i in range(3):
                nc.tensor.matmul(wu_ps[:], ones_sb[:], wu_sb[:],
                                 start=(i == 0), stop=(i == 2))
            wu_exp = wpool.tile([1, 8], bf16, tag="wux")
            nc.scalar.activation(wu_exp[:], wu_sb[0:1, 0:8], AF.Exp)

            pt = {}      # (b, h, kb, half) -> [128, 1024] exp'd scores
            opair = {}   # b -> [128, N] normalized o for the head pair
            OT = {}      # b -> [128, N] transposed (feature-major) o
            avt = {}     # (b, h, half) -> psum accumulator [128, 512]
            rec = {}     # (b, h) -> [128, 16] reciprocal denominators

            def emit_exp(dst, src, dve):
                """exp(src) -> dst (bf16).  dve=True uses the DVE fast-exp
                (bitcast Schraudolph); otherwise ScalarE's exact exp."""
                if dve:
                    nc.vector.tensor_scalar(
                        dst.bitcast(i16), src, SCHRAUD_A, SCHRAUD_B,
                        ALU.mult, ALU.add)
                else:
                    nc.scalar.activation(dst, src, AF.Exp)

            def emit_scores_half(b, h, kb, half):
                ps = psp.tile([128, 1024], f32, tag="ps",
                              name=f"ps{b}{h}_{kb}_{half}")
                for j in range(2):
                    qc = half * 2 + j
                    nc.tensor.matmul(
                        ps[:, j * 512:(j + 1) * 512],
                        KT[b][h * 64:(h + 1) * 64,
                              kb * 128:(kb + 1) * 128],
                        QT[b][h * 64:(h + 1) * 64,
                              qc * 512:(qc + 1) * 512],
                        start=True, stop=True)
                t = ptpool.tile([128, 1024], bf16, tag="pt",
                                name=f"pt{b}{h}_{kb}_{half}")
                emit_exp(t[:], ps[:], (2 * b + h, kb, half) in DVE_EXP)
                if kb in (0, 15) and half == 1:
                    probe(nc.scalar, f"exp_u{2 * b + h}_kb{kb}_{half}",
                          t[:, 0:1])
                pt[(b, h, kb, half)] = t
                probe(nc.tensor, f"scores_u{2 * b + h}_kb{kb}_{half}",
                      ps[:, 0:1])

            def emit_av_pass(b, h, qb):
                """AV (+denominator in column 64) for query block qb of head
                (b,h).  Normalization is batched separately (emit_norms)."""
                grp, sl = qb // 7, qb % 7   # 7 x 65 = 455 cols per psum bank
                half = qb // 8
                if sl == 0:
                    avt[(b, h, grp)] = avp.tile(
                        [128, 512], f32, tag="av", name=f"av{b}{h}_{grp}")
                if qb == 0:
                    rec[(b, h)] = recpool.tile(
                        [128, 16], f32, tag="rec", name=f"rec{b}{h}")
                    if h == 0:
                        opair[b] = opool.tile(
                            [128, N], bf16, tag="o", name=f"o{b}")
                av = avt[(b, h, grp)]
                for kb in range(NB):
                    nc.tensor.matmul(
                        av[:, sl * 65:(sl + 1) * 65],
                        pt[(b, h, kb, half)][:, (qb % 8) * 128:
                                             (qb % 8 + 1) * 128],
                        vone_ap(b, kb, h),
                        start=(kb == 0), stop=(kb == NB - 1))
                probe(nc.tensor, f"av_u{2 * b + h}_qb{qb}",
                      av[:, sl * 65:sl * 65 + 1])

            def _norm_one(b, h, qb):
                grp, sl = qb // 7, qb % 7
                av = avt[(b, h, grp)]
                r = rec[(b, h)]
                nc.vector.reciprocal(r[:, qb:qb + 1],
                                     av[:, sl * 65 + 64:sl * 65 + 65])
                dst = opair[b][:, qb * 128 + h * 64:qb * 128 + (h + 1) * 64]
                nc.vector.tensor_scalar_mul(
                    dst, av[:, sl * 65:sl * 65 + 64], r[:, qb:qb + 1])

            def emit_norms(b, h, qb1):
                """Normalize query blocks (qb1-1, qb1) of head (b,h) --
                one paired reciprocal + one paired multiply when both
                blocks share an AV psum group, two singles otherwise."""
                qb0 = qb1 - 1
                if qb0 // 7 != qb1 // 7:
                    _norm_one(b, h, qb0)
                    _norm_one(b, h, qb1)
                    return
                grp, sl = qb0 // 7, qb0 % 7
                av = avt[(b, h, grp)]
                r = rec[(b, h)]
                av_g = av[:, 0:455].rearrange("p (s e) -> p s e", e=65)
                nc.vector.reciprocal(r[:, qb0:qb0 + 2],
                                     av_g[:, sl:sl + 2, 64:65])
                dst = opair[b][:].rearrange("p (q f) -> p q f", f=128)[
                    :, qb0:qb0 + 2, h * 64:(h + 1) * 64]
                nc.vector.tensor_mul(
                    dst, av_g[:, sl:sl + 2, 0:64],
                    r[:, qb0:qb0 + 2].unsqueeze(2).to_broadcast((128, 2, 64)))

            def emit_transpose_pair(b, qb1, eng=None):
                """o^T for query blocks (qb1-1, qb1) in one xbar DMA."""
                if qb1 == 1:
                    OT[b] = otpool.tile([128, N], bf16, tag="ot",
                                        name=f"OT{b}")
                qb0 = qb1 - 1
                (eng or nc.sync).dma_start_transpose(
                    out=OT[b][:, qb0 * 128:(qb1 + 1) * 128].rearrange(
                        "p (j t) -> p j t", t=128),
                    in_=opair[b][:, qb0 * 128:(qb1 + 1) * 128])

            def emit_outproj_pair(b, pair, dma_eng, tail=False, last=False,
                                  act_copies=None):
                """Out-projection + output DMA for token blocks
                (2*pair, 2*pair+1)."""
                o = obpool.tile([128, 2 * C], bf16, tag="ob",
                                name=f"ob{b}_{pair}")
                pcs = []
                if tail and pair % 2:
                    big = psp.tile([128, 1024], f32, tag="ps",
                                   name=f"pcb{b}_{pair}")
                    pcs += [big[:, 0:512], big[:, 512:1024]]
                else:
                    pcs += [shp.tile([128, 512], f32, tag="sh",
                                     name=f"pc{b}_{pair}_{i}")[:]
                            for i in range(2)]
                if tail and pair % 2 == 0:
                    big = psp.tile([128, 1024], f32, tag="ps",
                                   name=f"pcb{b}_{pair}")
                    pcs += [big[:, 0:512], big[:, 512:1024]]
                else:
                    pcs += [shp.tile([128, 512], f32, tag="sh",
                                     name=f"pc{b}_{pair}_{i + 2}")[:]
                            for i in range(2)]
                for j in range(2):
                    blk = 2 * pair + j
                    for ofh in range(2):
                        pc = pcs[2 * j + ofh]
                        nc.tensor.matmul(pc,
                                         OT[b][:, blk * 128:(blk + 1) * 128],
                                         wo_sb[:, ofh * 512:(ofh + 1) * 512],
                                         start=True, stop=True)
                        # psum reads are only legal on PE/Act/DVE: copies ride
                        # DVE mid-kernel; in the tail idle ScalarE takes half
                        if act_copies is None:
                            on_act = tail and (ofh + j) % 2 == (
                                0 if last else 1)
                        else:
                            on_act = (2 * j + ofh) % 2 == 1 and \
                                (2 * j + ofh) // 2 < act_copies
                        if on_act:
                            nc.scalar.copy(
                                o[:, (2 * j + ofh) * 512:
                                  (2 * j + ofh + 1) * 512], pc)
                        else:
                            nc.vector.tensor_copy(
                                o[:, (2 * j + ofh) * 512:
                                  (2 * j + ofh + 1) * 512], pc)
                probe(nc.gpsimd, f"outproj_b{b}_pair{pair}", o[:, 0:1])
                if last:
                    # final pair: two single-block DMAs on different queues
                    # so block 14's transfer overlaps block 15's copies
                    for j in range(2):
                        (nc.gpsimd if j == 0 else nc.sync).dma_start(
                            out=out_p[b][(2 * pair + j) * 128:
                                         (2 * pair + j + 1) * 128, :],
                            in_=o[:, j * C:(j + 1) * C])
                else:
                    dma_eng.dma_start(
                        out=out_p[b].rearrange("(blk p) c -> p blk c", p=128)
                            [:, 2 * pair:2 * pair + 2, :],
                        in_=o[:].rearrange("p (blk c) -> p blk c", c=C))

            # b1 Q/K chunks spread over head-unit 1 (7 of 8; K ch3 lands
            # in unit 2 where there is slack)
            qkv_b1 = ([("q", ch) for ch in range(QC)]
                      + [("k", ch) for ch in range(QC - 1)])

            # remaining b0 K chunks, injected into early u0 units
            qk_b0_rest = [("k", 1), ("k", 2), ("k", 3)]

            def emit_extras(u, kb):
                """Non-score work scheduled alongside unit (u, kb)."""
                if u == 0:
                    if kb < len(qk_b0_rest):
                        kind, ch = qk_b0_rest[kb]
                        (emit_q_chunk if kind == "q" else emit_k_chunk)(
                            0, ch, shp, "sh")
                    else:
                        # V(b0): 16 blocks over units 3..15
                        lo = ((kb - 3) * 16) // 13
                        hi = ((kb - 2) * 16) // 13
                        for nb in range(lo, min(hi, 16)):
                            emit_v_block(0, nb, shp, "sh")
                    if kb >= 4:
                        for i in range(3 * (kb - 4), 3 * (kb - 3)):
                            if i < 32:
                                load_x(1, i % 8, i // 8, nc.sync)
                    if kb == 8:
                        alloc_proj(1)
                elif u == 1:
                    emit_av_pass(0, 0, kb)
                    if kb % 2:
                        emit_norms(0, 0, kb)
                    lo = (kb * 7) // 16
                    hi = ((kb + 1) * 7) // 16
                    for kind, idx in qkv_b1[lo:hi]:
                        if kind == "q":
                            emit_q_chunk(1, idx, shp, "sh")
                        else:
                            emit_k_chunk(1, idx, shp, "sh")
                elif u == 2:
                    emit_av_pass(0, 1, kb)
                    if kb % 2:
                        emit_norms(0, 1, kb)
                        emit_transpose_pair(0, kb)
                    if kb == 0:
                        emit_k_chunk(1, 3, shp, "sh")
                    emit_v_block(1, kb, shp, "sh")

                else:
                    emit_av_pass(1, 0, kb)
                    if kb % 2:
                        emit_norms(1, 0, kb)
                    # front-load b0's out-projection into the light early
                    # units so the stretch's tail stays at exp pace
                    if kb < 8:
                        emit_outproj_pair(0, kb,
                                          nc.gpsimd if kb % 2 else nc.sync)
                    # the last head's half0 exps are all done by step kb=8
                    # (its scores stream runs half0-first), so its first AV
                    # passes and out-projection start inside the stream
                    if kb >= 9:
                        qb = kb - 9
                        emit_av_pass(1, 1, qb)
                        if qb % 2:
                            emit_norms(1, 1, qb)
                            emit_transpose_pair(1, qb)
                            # out-projection lags the transposes by one
                            # pair so the chain stays pipelined
                            if qb >= 3:
                                emit_outproj_pair(
                                    1, (qb - 2) // 2,
                                    nc.sync if qb % 4 == 1 else nc.gpsimd)


            # score-half stream: one unit of lookahead vs companion work;
            # the LAST head emits all its half-0 scores before half-1 so
            # its AV can begin long before its final exp
            score_halves = []
            for b in range(B):
                for h in range(HPC):
                    if (b, h) == (B - 1, HPC - 1):
                        score_halves += [(b, h, kb, 0) for kb in range(NB)]
                        score_halves += [(b, h, kb, 1) for kb in range(NB)]
                    else:
                        score_halves += [(b, h, kb, half)
                                         for kb in range(NB)
                                         for half in range(2)]

            # prologue: minimum projection before the first score work.
            # The very first score half is split into two 512-wide pieces
            # so the exp stream starts after Q0+K0 alone (not Q1)
            emit_q_chunk(0, 0, shp, "sh")
            emit_k_chunk(0, 0, shp, "sh")
            ps0 = psp.tile([128, 1024], f32, tag="ps", name="ps_first")
            t0 = ptpool.tile([128, 1024], bf16, tag="pt", name="pt_first")
            nc.tensor.matmul(ps0[:, 0:512], KT[0][0:64, 0:128],
                             QT[0][0:64, 0:512], start=True, stop=True)
            nc.scalar.activation(t0[:, 0:512], ps0[:, 0:512], AF.Exp)
            emit_q_chunk(0, 1, shp, "sh")
            nc.tensor.matmul(ps0[:, 512:1024], KT[0][0:64, 0:128],
                             QT[0][0:64, 512:1024], start=True, stop=True)
            nc.scalar.activation(t0[:, 512:1024], ps0[:, 512:1024], AF.Exp)
            pt[(0, 0, 0, 0)] = t0
            emit_q_chunk(0, 2, shp, "sh")
            ps1 = psp.tile([128, 1024], f32, tag="ps", name="ps_second")
            t1 = ptpool.tile([128, 1024], bf16, tag="pt", name="pt_second")
            nc.tensor.matmul(ps1[:, 0:512], KT[0][0:64, 0:128],
                             QT[0][0:64, 1024:1536], start=True, stop=True)
            nc.scalar.activation(t1[:, 0:512], ps1[:, 0:512], AF.Exp)
            emit_q_chunk(0, 3, shp, "sh")
            nc.tensor.matmul(ps1[:, 512:1024], KT[0][0:64, 0:128],
                             QT[0][0:64, 1536:2048], start=True, stop=True)
            nc.scalar.activation(t1[:, 512:1024], ps1[:, 512:1024], AF.Exp)
            pt[(0, 0, 0, 1)] = t1

            steps = [(2 * b + h, kb) for b in range(B) for h in range(HPC)
                     for kb in range(NB)]
            for i, (u, kb) in enumerate(steps):
                for j in (2 * (i + 1), 2 * (i + 1) + 1):
                    if j < len(score_halves):
                        emit_scores_half(*score_halves[j])
                emit_extras(u, kb)

            # ---- tail: last head (b1, h1) remainder (qb 7..15) ----
            # qb7 (half0) is a normal pass; qb8-15 (half1) run STEP-MAJOR so
            # every pass only trails the final exp by its last kb step.
            for qb in range(7, NB):
                emit_av_pass(1, 1, qb)
                if qb % 2:
                    emit_norms(1, 1, qb)
                    emit_transpose_pair(1, qb)
                    emit_outproj_pair(1, (qb - 2) // 2,
                                      nc.sync if qb % 4 == 1 else nc.gpsimd,
                                      tail=True)
            emit_outproj_pair(1, 7, nc.gpsimd, tail=True, last=True)
    return nc


def _prep_in_maps(x, qkv_w, qkv_b, out_w):
    bf = ml_dtypes.bfloat16
    scale = 1.0 / np.sqrt(D)
    in_maps = []
    for c in range(NCORES):
        h0 = HPC * c
        qsl = slice(h0 * D, (h0 + HPC) * D)
        ksl = slice(C + h0 * D, C + (h0 + HPC) * D)
        vsl = slice(2 * C + h0 * D, 2 * C + (h0 + HPC) * D)
        in_maps.append({
            "xt0": np.ascontiguousarray(x[0].T).astype(bf),
            "xt1": np.ascontiguousarray(x[1].T).astype(bf),
            "wqT": np.ascontiguousarray((qkv_w[qsl] * scale).T).astype(bf),
            "wkT": np.ascontiguousarray(qkv_w[ksl].T).astype(bf),
            "wvT": np.ascontiguousarray(qkv_w[vsl].T).astype(bf),
            "woT": np.ascontiguousarray(out_w[:, h0 * D:(h0 + HPC) * D].T
                                        ).astype(bf),
            "bq": (qkv_b[qsl] * scale).reshape(128, 1).astype(np.float32),
            "bk": qkv_b[ksl].reshape(128, 1).astype(np.float32),
        })
    return in_maps


def kernel(x, qkv_w, qkv_b, out_w, out_b):
    from concourse.bass_utils import run_bass_kernel_spmd

    x = np.asarray(x, dtype=np.float32)
    qkv_w = np.asarray(qkv_w, dtype=np.float32)
    qkv_b = np.asarray(qkv_b, dtype=np.float32)
    out_w = np.asarray(out_w, dtype=np.float32)
    out_b = np.asarray(out_b, dtype=np.float32)

    if "nc" not in _cache:
        _cache["nc"] = _build()
    in_maps = _prep_in_maps(x, qkv_w, qkv_b, out_w)
    res = run_bass_kernel_spmd(_cache["nc"], in_maps, list(range(NCORES)))
    out = np.zeros((B, N, C), np.float32)
    for c in range(NCORES):
        for b in range(B):
            out[b] += res.results[c][f"out{b}"].astype(np.float32)
    out += (out_b + qkv_b[2 * C:] @ out_w.T)[None, None, :]
    return out
